# revision 48
# baseline (speedup 1.0000x reference)
"""GPT2 attention (B=2,S=2048,E=1024,H=16) on 8 NeuronCores.

Sharding: core c -> batch b=c//4, head-group g=c%4 (4 heads, d'=256 cols).

Structure (v2d): per-q-chunk pipeline. For each q-chunk qc (512 queries):
  - project Q,K for s-chunk qc and V for s-tiles 4qc..4qc+3 (causal)
  - attention for qc (scores -> exp -> mask -> attnV), head pairs
    interleaved at k-pair level
  - fast-drain ots PSUM to SBUF; normalize off the critical path
  - c_proj for the 4 s-tiles; bf16 partials summed on host
Inputs packed into few large DMAs (DMA issue costs ~0.6us each, serial).
Split PSUM pools so next-chunk projections don't queue behind c_proj.
"""

import numpy as np

import concourse.bass as bass
import concourse.mybir as mybir
import concourse.tile as tile
from concourse import bacc
from concourse.bass_utils import run_bass_kernel_spmd

B, S, E, H = 2, 2048, 1024, 16
HD = 64           # head dim
HPC = 4           # heads per core
DP = HPC * HD     # 256 d' columns per core
NQC = 4           # q-chunks of 512
QCW = 512         # q-chunk width
NKT = S // 128    # 16 k-tiles
NST = S // 128    # 16 s-tiles
NET = E // 128    # 8 E-tiles

f32 = mybir.dt.float32
bf16 = mybir.dt.bfloat16
FT = mybir.ActivationFunctionType

_CACHED = {}


def build_nc():
    nc = bacc.Bacc("TRN2", target_bir_lowering=False, debug=False,
                   enable_asserts=False, num_devices=8)

    # packed inputs: one DRAM tensor per packed SBUF tile
    xT = nc.dram_tensor("xT", [E, S], bf16, kind="ExternalInput")
    wqk8 = nc.dram_tensor("wqk8", [128, NET * 512], bf16, kind="ExternalInput")
    bqk = nc.dram_tensor("bqk", [128, 4], f32, kind="ExternalInput")
    wv8 = nc.dram_tensor("wv8", [128, NET * 260], bf16, kind="ExternalInput")
    vb = nc.dram_tensor("vb", [128, 260], f32, kind="ExternalInput")
    wp8 = nc.dram_tensor("wp8", [128, 2 * E], bf16, kind="ExternalInput")
    mask4 = nc.dram_tensor("mask4", [128, 2048], bf16, kind="ExternalInput")
    outp = nc.dram_tensor("outp", [S, E], bf16, kind="ExternalOutput")

    with tile.TileContext(nc) as tc:
        with (
            tc.tile_pool(name="consts", bufs=1) as consts,
            tc.tile_pool(name="acts", bufs=1) as acts,
            tc.tile_pool(name="slabs", bufs=5) as slabs,
            tc.tile_pool(name="obufs", bufs=4) as obufs,
            tc.tile_pool(name="small", bufs=3) as small,
            tc.tile_pool(name="outs", bufs=3) as outs,
            tc.tile_pool(name="spp", bufs=2, space="PSUM") as spp,
            tc.tile_pool(name="otp", bufs=2, space="PSUM") as otp,
            tc.tile_pool(name="pp", bufs=2, space="PSUM") as pp,
        ):
            # ---- inputs, few big DMAs, priority order ----
            # wqk and x chunk 0 split in halves so the first QK matmuls
            # can start as early as possible
            wqk_sb = consts.tile([128, NET * 512], bf16, tag="wqk")
            nc.sync.dma_start(wqk_sb[:, 0:4 * 512], wqk8[:, 0:4 * 512])
            # x: one big tile, E-tile t at cols [t*2048, (t+1)*2048);
            # loaded in s-chunk DMAs (strided dst across the 8 blocks)
            x_sb = consts.tile([128, NET * S], bf16, tag="x")
            xv = x_sb.rearrange("p (t f) -> p t f", t=NET)

            def xchunk(sc, ts=slice(0, NET), eng=None):
                (eng or nc.sync).dma_start(
                    xv[:, ts, sc * 512:(sc + 1) * 512],
                    xT[:, sc * 512:(sc + 1) * 512].rearrange(
                        "(t p) c -> p t c", p=128)[:, ts, :])

            xchunk(0, slice(0, 4), eng=nc.scalar)
            xchunk(0, slice(4, 8), eng=nc.scalar)
            nc.sync.dma_start(wqk_sb[:, 4 * 512:], wqk8[:, 4 * 512:])
            wv_sb = consts.tile([128, NET * 260], bf16, tag="wv")
            nc.sync.dma_start(wv_sb[:], wv8[:, :])
            bqk_sb = consts.tile([128, 4], f32, tag="bqk")
            nc.sync.dma_start(bqk_sb[:], bqk[:, :])
            vb_sb = consts.tile([128, 260], f32, tag="vb")
            nc.sync.dma_start(vb_sb[:], vb[:, :])
            mask4_sb = consts.tile([128, 2048], bf16, tag="mask4")
            nc.sync.dma_start(mask4_sb[:], mask4[:, :])
            xchunk(1)
            wp_sb = consts.tile([128, 2 * E], bf16, tag="wp")
            nc.sync.dma_start(wp_sb[:], wp8[:, :])
            xchunk(2)
            xchunk(3)

            def xt(t):
                return x_sb[:, t * S:(t + 1) * S]

            # ---- persistent activations ----
            v_sb = [acts.tile([128, 260], bf16, tag=f"v{st}", name=f"v{st}")
                    for st in range(NST)]
            qkt_sb = [acts.tile([128, S], bf16, tag=f"qkt{t}", name=f"qkt{t}")
                      for t in range(4)]
            attnT_sb = [acts.tile([128, S], bf16, tag=f"attnT{t}", name=f"attnT{t}")
                        for t in range(2)]

            def cproj(qc, final=False):
                # c_proj for the 4 s-tiles of q-chunk qc (issued one chunk
                # late so the PE never stalls on the normalize chain).
                # final: drain on the (idle) Scalar engine and DMA each half
                # as soon as it lands, to shorten the epilogue.
                for sti in range(4):
                    st = 4 * qc + sti
                    ob2 = outs.tile([128, 1024], bf16, tag="ob",
                                    name=f"ob{st}")
                    for nchk in range(2):
                        cps = pp.tile([128, 512], f32, tag="pp")
                        for kt2 in range(2):
                            nc.tensor.matmul(
                                cps[:],
                                attnT_sb[kt2][:, st * 128:(st + 1) * 128],
                                wp_sb[:, kt2 * E + nchk * 512:
                                      kt2 * E + (nchk + 1) * 512],
                                start=(kt2 == 0), stop=(kt2 == 1),
                            )
                        if final:
                            nc.scalar.activation(
                                ob2[:, nchk * 512:(nchk + 1) * 512],
                                cps[:], FT.Copy)
                            nc.sync.dma_start(
                                outp[st * 128:(st + 1) * 128,
                                     nchk * 512:(nchk + 1) * 512],
                                ob2[:, nchk * 512:(nchk + 1) * 512])
                        else:
                            nc.vector.tensor_copy(
                                ob2[:, nchk * 512:(nchk + 1) * 512], cps[:])
                    if not final:
                        nc.sync.dma_start(
                            outp[st * 128:(st + 1) * 128, :], ob2[:])

            def qkproj(nq, t):
                qps = pp.tile([128, 512], f32, tag="pp")
                for kt in range(NET):
                    nc.tensor.matmul(
                        qps[:],
                        wqk_sb[:, kt * 512 + t * 128:kt * 512 + (t + 1) * 128],
                        xt(kt)[:, nq * 512:(nq + 1) * 512],
                        start=(kt == 0), stop=(kt == NET - 1),
                    )
                nc.vector.tensor_scalar_add(
                    qkt_sb[t][:, nq * 512:(nq + 1) * 512],
                    qps[:], bqk_sb[:, t:t + 1])

            def vproj(st):
                vps = pp.tile([128, 512], f32, tag="pp")
                for kt in range(NET):
                    nc.tensor.matmul(
                        vps[:, 0:260],
                        xt(kt)[:, st * 128:(st + 1) * 128],
                        wv_sb[:, kt * 260:(kt + 1) * 260],
                        start=(kt == 0), stop=(kt == NET - 1),
                    )
                nc.vector.tensor_add(v_sb[st][:], vps[:, 0:260], vb_sb[:])

            # prologue: projections for chunk 0
            for t in (0, 2, 1, 3):
                qkproj(0, t)
            for st in range(4):
                vproj(st)

            for qc in range(NQC):
                nkt = 4 * qc + 4  # causal: k-tiles 0 .. 4qc+3

                # ---- attention: head pairs interleaved at k-pair level;
                # previous chunk's c_proj issued between the two head pairs
                # so its matmuls stay available as late PE filler ----
                for hp in range(2):
                    if hp == 1 and qc > 0:
                        cproj(qc - 1)
                    ots = [otp.tile([65, 512], f32, tag="ot",
                                    name=f"ot{qc}_{hp}_{i}") for i in range(2)]
                    pend = None

                    def attnv(pend):
                        kts_p, los_p, slabs_p = pend
                        for half in range(2):
                            kt, lo = kts_p[half], los_p[half]
                            for i in range(2):
                                h = 2 * hp + i
                                nc.tensor.matmul(
                                    ots[i][:, lo:512],
                                    v_sb[kt][:, 65 * h:65 * h + 65],
                                    slabs_p[i][:, half * 512 + lo:
                                               (half + 1) * 512],
                                    start=(kt == 0), stop=(kt == nkt - 1),
                                )

                    for kp in range(nkt // 2):  # k-tile pairs
                        # lo(kt): first query column (within the chunk) that
                        # k-tile kt can contribute to (causal). Scores, exp
                        # and attnV are all narrowed to [lo, 512); only the
                        # [128,128] boundary triangle needs a mask multiply.
                        kts = (2 * kp, 2 * kp + 1)
                        los = [max(0, 128 * (kt - 4 * qc)) for kt in kts]
                        # scores for both heads interleaved so consecutive
                        # matmuls hit alternating PE row groups (po 0/64)
                        sps = [spp.tile([128, 1024], f32, tag="sp",
                                        name=f"sp{qc}_{hp}_{kp}_{i}")
                               for i in range(2)]
                        for half in range(2):
                            kt, lo = kts[half], los[half]
                            for i in range(2):
                                h = 2 * hp + i
                                tq = h // 2
                                po = (h % 2) * 64
                                nc.tensor.matmul(
                                    sps[i][:, half * 512 + lo:
                                           (half + 1) * 512],
                                    qkt_sb[2 + tq][po:po + 64,
                                                   kt * 128:(kt + 1) * 128],
                                    qkt_sb[tq][po:po + 64,
                                               qc * 512 + lo:(qc + 1) * 512],
                                    start=True, stop=True,
                                )
                        slabs2 = []
                        for i in range(2):
                            slab = slabs.tile([128, 1024], bf16, tag="slab",
                                              name=f"slab{qc}_{hp}_{kp}_{i}")
                            if los[1] == 0:
                                nc.scalar.activation(slab[:], sps[i][:], FT.Exp)
                            else:
                                nc.scalar.activation(
                                    slab[:, los[0]:512],
                                    sps[i][:, los[0]:512], FT.Exp)
                                nc.scalar.activation(
                                    slab[:, 512 + los[1]:1024],
                                    sps[i][:, 512 + los[1]:1024], FT.Exp)
                            for half in range(2):
                                di = kts[half] - 4 * qc
                                if di >= 0:  # boundary triangle only
                                    c0 = half * 512 + 128 * di
                                    nc.gpsimd.tensor_mul(
                                        slab[:, c0:c0 + 128],
                                        slab[:, c0:c0 + 128],
                                        mask4_sb[:, di * 512 + di * 128:
                                                 di * 512 + (di + 1) * 128])
                            slabs2.append(slab)
                        if pend is not None:
                            attnv(pend)
                        pend = (kts, los, slabs2)
                    attnv(pend)
                    if hp == 1 and qc < NQC - 1:
                        # next chunk's QK projection issued before this
                        # normalize so its drains unblock scores first
                        for t in (0, 2, 1, 3):
                            qkproj(qc + 1, t)
                    # fast-drain PSUM to SBUF, normalize off the critical path.
                    # Final chunk: normalize in q-halves so c_proj of the
                    # first two s-tiles starts while the rest still drains.
                    obs = []
                    for i in range(2):
                        ob = obufs.tile([65, 512], f32, tag="ob",
                                        name=f"ob{qc}_{hp}_{i}")
                        nc.vector.tensor_copy(ob[:], ots[i][:])
                        obs.append(ob)
                    chs = ((0, 256), (256, 512)) if qc == NQC - 1 else ((0, 512),)
                    for c0, c1 in chs:
                        for i in range(2):
                            h = 2 * hp + i
                            po = (h % 2) * 64
                            ob = obs[i]
                            zrow = small.tile([1, 512], f32, tag="zrow")
                            nc.vector.tensor_copy(
                                zrow[:, c0:c1], ob[64:65, c0:c1])
                            rz = small.tile([1, 512], f32, tag="rz")
                            nc.vector.reciprocal_approx_fast(
                                rz[:, c0:c1], zrow[:, c0:c1])
                            sbb = small.tile([64, 512], f32, tag="sbb")
                            nc.gpsimd.partition_broadcast(
                                sbb[:, c0:c1], rz[0:1, c0:c1])
                            nc.vector.tensor_mul(
                                attnT_sb[h // 2][po:po + 64,
                                                 qc * 512 + c0:qc * 512 + c1],
                                ob[0:64, c0:c1], sbb[:, c0:c1])
                    if hp == 1 and qc < NQC - 1:
                        for sti in range(4):
                            vproj(4 * (qc + 1) + sti)

            cproj(NQC - 1, final=True)

    nc.finalize()
    return nc


def _prep_inputs(hidden_states, w_attn, b_attn, w_proj, b_proj):
    hs = np.asarray(hidden_states, np.float32)
    wa = np.asarray(w_attn, np.float32)
    ba = np.asarray(b_attn, np.float32)
    wpj = np.asarray(w_proj, np.float32)

    import ml_dtypes
    bfl = ml_dtypes.bfloat16
    xTs = [np.ascontiguousarray(hs[b].T.astype(bfl)) for b in range(B)]
    triu = (np.arange(128)[:, None] <= np.arange(128)[None, :]).astype(np.float32)
    mask4 = np.zeros((128, 2048), np.float32)
    for i in range(4):
        m = np.ones((128, 512), np.float32)
        m[:, :i * 128] = 0.0
        m[:, i * 128:(i + 1) * 128] = triu
        mask4[:, i * 512:(i + 1) * 512] = m
    mask4 = mask4.astype(ml_dtypes.bfloat16)

    in_maps = []
    for c in range(8):
        b, g = c // 4, c % 4
        q0 = DP * g
        k0 = E + DP * g
        v0 = 2 * E + DP * g
        wqk = np.concatenate(
            [wa[:, q0:q0 + DP] * 0.125, wa[:, k0:k0 + DP]], axis=1).astype(bfl)
        # pack [E, 512] -> [128, 8*512] (E-tile t at cols t*512)
        wqk8 = np.ascontiguousarray(
            wqk.reshape(NET, 128, 512).transpose(1, 0, 2).reshape(128, NET * 512))
        bqk = np.zeros((128, 4), np.float32)
        bqk[:, 0] = ba[q0:q0 + 128] * 0.125
        bqk[:, 1] = ba[q0 + 128:q0 + 256] * 0.125
        bqk[:, 2] = ba[k0:k0 + 128]
        bqk[:, 3] = ba[k0 + 128:k0 + 256]
        wv = np.zeros((E, 260), bfl)
        vb = np.zeros((128, 260), np.float32)
        for h in range(HPC):
            wv[:, 65 * h:65 * h + 64] = wa[:, v0 + 64 * h:v0 + 64 * h + 64].astype(bfl)
            vb[:, 65 * h:65 * h + 64] = ba[v0 + 64 * h:v0 + 64 * h + 64]
            vb[:, 65 * h + 64] = 1.0
        wv8 = np.ascontiguousarray(
            wv.reshape(NET, 128, 260).transpose(1, 0, 2).reshape(128, NET * 260))
        wp = wpj[DP * g:DP * (g + 1), :].astype(bfl)
        wp8 = np.ascontiguousarray(
            wp.reshape(2, 128, E).transpose(1, 0, 2).reshape(128, 2 * E))
        in_maps.append({
            "xT": xTs[b],
            "wqk8": wqk8,
            "bqk": bqk,
            "wv8": wv8,
            "vb": vb,
            "wp8": wp8,
            "mask4": mask4,
        })
    return in_maps


def run(trace=False, **inputs):
    if "nc" not in _CACHED:
        _CACHED["nc"] = build_nc()
    nc = _CACHED["nc"]
    in_maps = _prep_inputs(**inputs)
    res = run_bass_kernel_spmd(nc, in_maps, list(range(8)), trace=trace)
    b_proj = np.asarray(inputs["b_proj"], np.float32)
    out = np.empty((B, S, E), np.float32)
    for b in range(B):
        acc = res.results[4 * b]["outp"].astype(np.float32)
        for g in range(1, 4):
            acc = acc + res.results[4 * b + g]["outp"].astype(np.float32)
        out[b] = acc + b_proj
    return out, res


def kernel(**inputs):
    out, _ = run(trace=False, **inputs)
    return out


# revision 49
# speedup vs baseline: 1.5594x; 1.5594x over previous
"""GPT2 attention (B=2,S=2048,E=1024,H=16) on 8 NeuronCores.

Sharding: core c -> batch b=c//4, head-group g=c%4 (4 heads, d'=256 cols).

Structure (v2d): per-q-chunk pipeline. For each q-chunk qc (512 queries):
  - project Q,K for s-chunk qc and V for s-tiles 4qc..4qc+3 (causal)
  - attention for qc (scores -> exp -> mask -> attnV), head pairs
    interleaved at k-pair level
  - fast-drain ots PSUM to SBUF; normalize off the critical path
  - c_proj for the 4 s-tiles; bf16 partials summed on host
Inputs packed into few large DMAs (DMA issue costs ~0.6us each, serial).
Split PSUM pools so next-chunk projections don't queue behind c_proj.
"""

import numpy as np

import concourse.bass as bass
import concourse.mybir as mybir
import concourse.tile as tile
from concourse import bacc
from concourse.bass_utils import run_bass_kernel_spmd

B, S, E, H = 2, 2048, 1024, 16
HD = 64           # head dim
HPC = 4           # heads per core
DP = HPC * HD     # 256 d' columns per core
NQC = 4           # q-chunks of 512
QCW = 512         # q-chunk width
NKT = S // 128    # 16 k-tiles
NST = S // 128    # 16 s-tiles
NET = E // 128    # 8 E-tiles

f32 = mybir.dt.float32
bf16 = mybir.dt.bfloat16
FT = mybir.ActivationFunctionType

_CACHED = {}


def build_nc():
    nc = bacc.Bacc("TRN2", target_bir_lowering=False, debug=False,
                   enable_asserts=False, num_devices=8)

    # packed inputs: one DRAM tensor per packed SBUF tile
    xT = nc.dram_tensor("xT", [E, S], bf16, kind="ExternalInput")
    wqk8 = nc.dram_tensor("wqk8", [128, NET * 512], bf16, kind="ExternalInput")
    bqk = nc.dram_tensor("bqk", [128, 4], f32, kind="ExternalInput")
    wv8 = nc.dram_tensor("wv8", [128, NET * 260], bf16, kind="ExternalInput")
    vb = nc.dram_tensor("vb", [128, 260], f32, kind="ExternalInput")
    wp8 = nc.dram_tensor("wp8", [128, 2 * E], bf16, kind="ExternalInput")
    mask4 = nc.dram_tensor("mask4", [128, 2048], bf16, kind="ExternalInput")
    outp = nc.dram_tensor("outp", [S, E], bf16, kind="ExternalOutput")

    with tile.TileContext(nc) as tc:
        with (
            tc.tile_pool(name="consts", bufs=1) as consts,
            tc.tile_pool(name="acts", bufs=1) as acts,
            tc.tile_pool(name="slabs", bufs=5) as slabs,
            tc.tile_pool(name="obufs", bufs=4) as obufs,
            tc.tile_pool(name="small", bufs=3) as small,
            tc.tile_pool(name="outs", bufs=3) as outs,
            tc.tile_pool(name="spp", bufs=2, space="PSUM") as spp,
            tc.tile_pool(name="otp", bufs=2, space="PSUM") as otp,
            tc.tile_pool(name="pp", bufs=2, space="PSUM") as pp,
        ):
            # ---- inputs, few big DMAs, priority order ----
            # wqk and x chunk 0 split in halves so the first QK matmuls
            # can start as early as possible
            wqk_sb = consts.tile([128, NET * 512], bf16, tag="wqk")
            nc.sync.dma_start(wqk_sb[:, 0:4 * 512], wqk8[:, 0:4 * 512])
            # x: one big tile, E-tile t at cols [t*2048, (t+1)*2048);
            # loaded in s-chunk DMAs (strided dst across the 8 blocks)
            x_sb = consts.tile([128, NET * S], bf16, tag="x")
            xv = x_sb.rearrange("p (t f) -> p t f", t=NET)

            def xchunk(sc, ts=slice(0, NET), eng=None):
                (eng or nc.sync).dma_start(
                    xv[:, ts, sc * 512:(sc + 1) * 512],
                    xT[:, sc * 512:(sc + 1) * 512].rearrange(
                        "(t p) c -> p t c", p=128)[:, ts, :])

            xchunk(0, slice(0, 4), eng=nc.scalar)
            xchunk(0, slice(4, 8), eng=nc.scalar)
            nc.sync.dma_start(wqk_sb[:, 4 * 512:], wqk8[:, 4 * 512:])
            wv_sb = consts.tile([128, NET * 260], bf16, tag="wv")
            nc.sync.dma_start(wv_sb[:], wv8[:, :])
            bqk_sb = consts.tile([128, 4], f32, tag="bqk")
            nc.sync.dma_start(bqk_sb[:], bqk[:, :])
            vb_sb = consts.tile([128, 260], f32, tag="vb")
            nc.sync.dma_start(vb_sb[:], vb[:, :])
            mask4_sb = consts.tile([128, 2048], bf16, tag="mask4")
            nc.sync.dma_start(mask4_sb[:], mask4[:, :])
            xchunk(1)
            wp_sb = consts.tile([128, 2 * E], bf16, tag="wp")
            nc.sync.dma_start(wp_sb[:], wp8[:, :])
            xchunk(2)
            xchunk(3)

            def xt(t):
                return x_sb[:, t * S:(t + 1) * S]

            # ---- persistent activations ----
            v_sb = [acts.tile([128, 260], bf16, tag=f"v{st}", name=f"v{st}")
                    for st in range(NST)]
            qkt_sb = [acts.tile([128, S], bf16, tag=f"qkt{t}", name=f"qkt{t}")
                      for t in range(4)]
            attnT_sb = [acts.tile([128, S], bf16, tag=f"attnT{t}", name=f"attnT{t}")
                        for t in range(2)]

            def cproj(qc, final=False):
                # c_proj for the 4 s-tiles of q-chunk qc (issued one chunk
                # late so the PE never stalls on the normalize chain).
                # final: drain on the (idle) Scalar engine and DMA each half
                # as soon as it lands, to shorten the epilogue.
                for sti in range(4):
                    st = 4 * qc + sti
                    ob2 = outs.tile([128, 1024], bf16, tag="ob",
                                    name=f"ob{st}")
                    for nchk in range(2):
                        cps = pp.tile([128, 512], f32, tag="pp")
                        for kt2 in range(2):
                            nc.tensor.matmul(
                                cps[:],
                                attnT_sb[kt2][:, st * 128:(st + 1) * 128],
                                wp_sb[:, kt2 * E + nchk * 512:
                                      kt2 * E + (nchk + 1) * 512],
                                start=(kt2 == 0), stop=(kt2 == 1),
                            )
                        if final:
                            nc.scalar.activation(
                                ob2[:, nchk * 512:(nchk + 1) * 512],
                                cps[:], FT.Copy)
                            nc.sync.dma_start(
                                outp[st * 128:(st + 1) * 128,
                                     nchk * 512:(nchk + 1) * 512],
                                ob2[:, nchk * 512:(nchk + 1) * 512])
                        else:
                            nc.vector.tensor_copy(
                                ob2[:, nchk * 512:(nchk + 1) * 512], cps[:])
                    if not final:
                        nc.sync.dma_start(
                            outp[st * 128:(st + 1) * 128, :], ob2[:])

            def qkproj(nq, t):
                qps = pp.tile([128, 512], f32, tag="pp")
                for kt in range(NET):
                    nc.tensor.matmul(
                        qps[:],
                        wqk_sb[:, kt * 512 + t * 128:kt * 512 + (t + 1) * 128],
                        xt(kt)[:, nq * 512:(nq + 1) * 512],
                        start=(kt == 0), stop=(kt == NET - 1),
                    )
                nc.vector.tensor_scalar_add(
                    qkt_sb[t][:, nq * 512:(nq + 1) * 512],
                    qps[:], bqk_sb[:, t:t + 1])

            def vproj(st):
                vps = pp.tile([128, 512], f32, tag="pp")
                for kt in range(NET):
                    nc.tensor.matmul(
                        vps[:, 0:260],
                        xt(kt)[:, st * 128:(st + 1) * 128],
                        wv_sb[:, kt * 260:(kt + 1) * 260],
                        start=(kt == 0), stop=(kt == NET - 1),
                    )
                nc.vector.tensor_add(v_sb[st][:], vps[:, 0:260], vb_sb[:])

            # prologue: projections for chunk 0
            for t in (0, 2, 1, 3):
                qkproj(0, t)
            for st in range(4):
                vproj(st)

            for qc in range(NQC):
                nkt = 4 * qc + 4  # causal: k-tiles 0 .. 4qc+3

                # ---- attention: head pairs interleaved at k-pair level;
                # previous chunk's c_proj issued between the two head pairs
                # so its matmuls stay available as late PE filler ----
                for hp in range(2):
                    if hp == 1 and qc > 0:
                        cproj(qc - 1)
                    ots = [otp.tile([65, 512], f32, tag="ot",
                                    name=f"ot{qc}_{hp}_{i}") for i in range(2)]
                    pend = None

                    def attnv(pend):
                        kts_p, los_p, slabs_p = pend
                        for half in range(2):
                            kt, lo = kts_p[half], los_p[half]
                            for i in range(2):
                                h = 2 * hp + i
                                nc.tensor.matmul(
                                    ots[i][:, lo:512],
                                    v_sb[kt][:, 65 * h:65 * h + 65],
                                    slabs_p[i][:, half * 512 + lo:
                                               (half + 1) * 512],
                                    start=(kt == 0), stop=(kt == nkt - 1),
                                )

                    for kp in range(nkt // 2):  # k-tile pairs
                        # lo(kt): first query column (within the chunk) that
                        # k-tile kt can contribute to (causal). Scores, exp
                        # and attnV are all narrowed to [lo, 512); only the
                        # [128,128] boundary triangle needs a mask multiply.
                        kts = (2 * kp, 2 * kp + 1)
                        los = [max(0, 128 * (kt - 4 * qc)) for kt in kts]
                        # scores for both heads interleaved so consecutive
                        # matmuls hit alternating PE row groups (po 0/64)
                        sps = [spp.tile([128, 1024], f32, tag="sp",
                                        name=f"sp{qc}_{hp}_{kp}_{i}")
                               for i in range(2)]
                        for half in range(2):
                            kt, lo = kts[half], los[half]
                            for i in range(2):
                                h = 2 * hp + i
                                tq = h // 2
                                po = (h % 2) * 64
                                nc.tensor.matmul(
                                    sps[i][:, half * 512 + lo:
                                           (half + 1) * 512],
                                    qkt_sb[2 + tq][po:po + 64,
                                                   kt * 128:(kt + 1) * 128],
                                    qkt_sb[tq][po:po + 64,
                                               qc * 512 + lo:(qc + 1) * 512],
                                    start=True, stop=True,
                                )
                        slabs2 = []
                        for i in range(2):
                            slab = slabs.tile([128, 1024], bf16, tag="slab",
                                              name=f"slab{qc}_{hp}_{kp}_{i}")
                            if los[1] == 0:
                                nc.scalar.activation(slab[:], sps[i][:], FT.Exp)
                            else:
                                nc.scalar.activation(
                                    slab[:, los[0]:512],
                                    sps[i][:, los[0]:512], FT.Exp)
                                nc.scalar.activation(
                                    slab[:, 512 + los[1]:1024],
                                    sps[i][:, 512 + los[1]:1024], FT.Exp)
                            for half in range(2):
                                di = kts[half] - 4 * qc
                                if di >= 0:  # boundary triangle only
                                    c0 = half * 512 + 128 * di
                                    nc.vector.tensor_mul(
                                        slab[:, c0:c0 + 128],
                                        slab[:, c0:c0 + 128],
                                        mask4_sb[:, di * 512 + di * 128:
                                                 di * 512 + (di + 1) * 128])
                            slabs2.append(slab)
                        if pend is not None:
                            attnv(pend)
                        pend = (kts, los, slabs2)
                    attnv(pend)
                    if hp == 1 and qc < NQC - 1:
                        # next chunk's QK projection issued before this
                        # normalize so its drains unblock scores first
                        for t in (0, 2, 1, 3):
                            qkproj(qc + 1, t)
                    # fast-drain PSUM to SBUF, normalize off the critical path.
                    # Final chunk: normalize in q-halves so c_proj of the
                    # first two s-tiles starts while the rest still drains.
                    obs = []
                    for i in range(2):
                        ob = obufs.tile([65, 512], f32, tag="ob",
                                        name=f"ob{qc}_{hp}_{i}")
                        nc.vector.tensor_copy(ob[:], ots[i][:])
                        obs.append(ob)
                    chs = ((0, 256), (256, 512)) if qc == NQC - 1 else ((0, 512),)
                    for c0, c1 in chs:
                        for i in range(2):
                            h = 2 * hp + i
                            po = (h % 2) * 64
                            ob = obs[i]
                            zrow = small.tile([1, 512], f32, tag="zrow")
                            nc.vector.tensor_copy(
                                zrow[:, c0:c1], ob[64:65, c0:c1])
                            rz = small.tile([1, 512], f32, tag="rz")
                            nc.vector.reciprocal_approx_fast(
                                rz[:, c0:c1], zrow[:, c0:c1])
                            sbb = small.tile([64, 512], f32, tag="sbb")
                            nc.gpsimd.partition_broadcast(
                                sbb[:, c0:c1], rz[0:1, c0:c1])
                            nc.vector.tensor_mul(
                                attnT_sb[h // 2][po:po + 64,
                                                 qc * 512 + c0:qc * 512 + c1],
                                ob[0:64, c0:c1], sbb[:, c0:c1])
                    if hp == 1 and qc < NQC - 1:
                        for sti in range(4):
                            vproj(4 * (qc + 1) + sti)

            cproj(NQC - 1, final=True)

    nc.finalize()
    return nc


def _prep_inputs(hidden_states, w_attn, b_attn, w_proj, b_proj):
    hs = np.asarray(hidden_states, np.float32)
    wa = np.asarray(w_attn, np.float32)
    ba = np.asarray(b_attn, np.float32)
    wpj = np.asarray(w_proj, np.float32)

    import ml_dtypes
    bfl = ml_dtypes.bfloat16
    xTs = [np.ascontiguousarray(hs[b].T.astype(bfl)) for b in range(B)]
    triu = (np.arange(128)[:, None] <= np.arange(128)[None, :]).astype(np.float32)
    mask4 = np.zeros((128, 2048), np.float32)
    for i in range(4):
        m = np.ones((128, 512), np.float32)
        m[:, :i * 128] = 0.0
        m[:, i * 128:(i + 1) * 128] = triu
        mask4[:, i * 512:(i + 1) * 512] = m
    mask4 = mask4.astype(ml_dtypes.bfloat16)

    in_maps = []
    for c in range(8):
        b, g = c // 4, c % 4
        q0 = DP * g
        k0 = E + DP * g
        v0 = 2 * E + DP * g
        wqk = np.concatenate(
            [wa[:, q0:q0 + DP] * 0.125, wa[:, k0:k0 + DP]], axis=1).astype(bfl)
        # pack [E, 512] -> [128, 8*512] (E-tile t at cols t*512)
        wqk8 = np.ascontiguousarray(
            wqk.reshape(NET, 128, 512).transpose(1, 0, 2).reshape(128, NET * 512))
        bqk = np.zeros((128, 4), np.float32)
        bqk[:, 0] = ba[q0:q0 + 128] * 0.125
        bqk[:, 1] = ba[q0 + 128:q0 + 256] * 0.125
        bqk[:, 2] = ba[k0:k0 + 128]
        bqk[:, 3] = ba[k0 + 128:k0 + 256]
        wv = np.zeros((E, 260), bfl)
        vb = np.zeros((128, 260), np.float32)
        for h in range(HPC):
            wv[:, 65 * h:65 * h + 64] = wa[:, v0 + 64 * h:v0 + 64 * h + 64].astype(bfl)
            vb[:, 65 * h:65 * h + 64] = ba[v0 + 64 * h:v0 + 64 * h + 64]
            vb[:, 65 * h + 64] = 1.0
        wv8 = np.ascontiguousarray(
            wv.reshape(NET, 128, 260).transpose(1, 0, 2).reshape(128, NET * 260))
        wp = wpj[DP * g:DP * (g + 1), :].astype(bfl)
        wp8 = np.ascontiguousarray(
            wp.reshape(2, 128, E).transpose(1, 0, 2).reshape(128, 2 * E))
        in_maps.append({
            "xT": xTs[b],
            "wqk8": wqk8,
            "bqk": bqk,
            "wv8": wv8,
            "vb": vb,
            "wp8": wp8,
            "mask4": mask4,
        })
    return in_maps


def run(trace=False, **inputs):
    if "nc" not in _CACHED:
        _CACHED["nc"] = build_nc()
    nc = _CACHED["nc"]
    in_maps = _prep_inputs(**inputs)
    res = run_bass_kernel_spmd(nc, in_maps, list(range(8)), trace=trace)
    b_proj = np.asarray(inputs["b_proj"], np.float32)
    out = np.empty((B, S, E), np.float32)
    for b in range(B):
        acc = res.results[4 * b]["outp"].astype(np.float32)
        for g in range(1, 4):
            acc = acc + res.results[4 * b + g]["outp"].astype(np.float32)
        out[b] = acc + b_proj
    return out, res


def kernel(**inputs):
    out, _ = run(trace=False, **inputs)
    return out
